# revision 33
# baseline (speedup 1.0000x reference)
"""Bass/Trainium2 kernel for masked attention + resize (nn_BaseAttender).

Full-input contract: kernel(**inputs) takes the complete unsharded tensors,
shards batch-wise across 8 NeuronCores (2 batches per core), runs one SPMD
Bass program, and gathers the full [16, 1024, 256] output.

Math (per batch):
    logits  = Q @ K^T / sqrt(512)              [1024, 2048]
    attn    = softmax(where(mask==0, -1e9, logits))
    out     = attn @ V @ W^T + b               [1024, 256]

v12 design (transposed scores + folded resize + scheduled loads):
  - scores are computed TRANSPOSED: S^T[k, q] with K chunks stationary and
    Q^T moving, so exp() lands directly in the [k, q] layout that the
    context matmul needs as stationary.  No DMA-xbar transposes at all.
  - W_resize is folded into V first: VW = V @ W^T ([2048, 256], 16K PE
    cycles), then context = attn @ VW (33K cycles) replaces attn @ V (66K)
    + ctx @ W (8K).  Saves ~24K PE cycles/batch and phase 3 entirely.
  - VW carries an extra ones column (moving width 257): the softmax
    denominator sum_k exp[q,k] accumulates in the same PSUM tile as the
    context, so no accum_out / cross-partition reduction is needed.
  - softmax without max-subtraction: logits are O(5) so exp() is safe, and
    where(mask==0,-1e9) + softmax == exp(logits + maskbias) / rowsum with
    maskbias = (mask-1)*28672 in fp8e5 (exp underflows to exactly 0).
  - all SBUF slot sizes are multiples of 1024B plus a leading 896B pad
    tile, so every moving-operand row sits on a 1024B boundary (the
    framework preamble otherwise leaves slots at 128 mod 1024).
  - phase 1 sweeps qc-outer/kt-inner with per-(kt,qc) single-bank PSUM
    tiles, so the PE can start after only ktr chunk0 + Q half0 (~1MB) has
    landed; the remaining K/mask chunks stream in deadline-ordered 0.25MB
    pieces round-robined over the three DMA queues (sync/scalar hwdge +
    gpsimd swdge), which each sustain only ~100-130 GB/s when all active.
  - the tile scheduler re-orders PE work using its own optimistic DMA
    model and otherwise hoists the VW phase to PE position ~16, where the
    in-order PE queue then serializes on the real (late) V arrival;
    tc.tile_wait_until pins VW after ph1 in the simulated timeline.
  - epilogue: 1/den * ctx + bias fused in one scalar_tensor_tensor per
    q-tile, streamed out per q-tile pair; the last pair stores per-q-tile
    on two queues in parallel.
  - measured (8-core SPMD, full inputs->full output): ~121us in the
    device's fast clock state, ~143us when the chip sits at 5/6 clock
    (global DVFS state, identical NEFF and instruction timeline).
"""

import sys

sys.path.insert(0, "/opt/trn_rl_repo")

import numpy as np
import ml_dtypes

import concourse.tile as tile
from concourse import bacc, mybir
from concourse.bass_utils import run_bass_kernel_spmd

# problem shape (hardcoded per contract)
B, NQ, NK, D, V, O = 16, 1024, 2048, 512, 512, 256
N_CORES = 8
B_LOC = B // N_CORES          # batches per core
SCALE = 1.0 / np.sqrt(np.float32(512.0))

P = 128
DT = D // P                   # 4 d-tiles (score contraction)
KT = NK // P                  # 16 k-tiles
QT = NQ // P                  # 8 q-tiles
QC = NQ // 512                # 2 q-chunks of 512 (score moving dim)
VT = V // P                   # 4 v-tiles (VW contraction)

F32 = mybir.dt.float32
BF = mybir.dt.bfloat16
E5 = mybir.dt.float8e5
U8 = mybir.dt.uint8

_NC_CACHE = {}


def _build():
    nc = bacc.Bacc(num_swdge_queues=2)
    # host-packed operands (see kernel() for exact packing)
    ktr = nc.declare_dram_parameter("ktr", [B_LOC, P, KT, DT, P], BF, isOutput=False)
    qtr = nc.declare_dram_parameter("qtr", [B_LOC, P, QC, DT, 512], BF, isOutput=False)
    vtr = nc.declare_dram_parameter("vtr", [B_LOC, P, VT, NK], BF, isOutput=False)
    msk = nc.declare_dram_parameter("msk", [B_LOC, P, KT, NQ], E5, isOutput=False)
    wtr = nc.declare_dram_parameter("wtr", [P, VT, O], BF, isOutput=False)
    b_r = nc.declare_dram_parameter("b_resize", [P, O], F32, isOutput=False)
    out = nc.declare_dram_parameter("out", [B_LOC, NQ, O], F32, isOutput=True)

    with tile.TileContext(nc) as tc:
        with (
            tc.tile_pool(name="const", bufs=1) as constp,
            tc.tile_pool(name="kt_sb", bufs=2) as ktp,
            tc.tile_pool(name="qt_sb", bufs=2) as qtp,
            tc.tile_pool(name="v_sb", bufs=2) as vp,
            tc.tile_pool(name="mrow", bufs=2) as mp,
            tc.tile_pool(name="expt", bufs=1) as etp,
            tc.tile_pool(name="vw", bufs=2) as vwp,
            tc.tile_pool(name="den", bufs=2) as dnp,
            tc.tile_pool(name="outsb", bufs=2) as osp,
            tc.tile_pool(name="ps1", bufs=5, space="PSUM") as psp1,   # [P,512] x5
            tc.tile_pool(name="ps2", bufs=3, space="PSUM") as psp2,   # [P,512] x3
        ):
            # alignment pad: preamble consts leave sbuf_base at 128 (mod 1024);
            # this tile shifts every later slot (all 1024B multiples) to 0.
            pad_sb = constp.tile([P, 896], U8)
            wt_sb = constp.tile([P, VT, O], BF)     # [v=128, vt, o]
            bias_sb = constp.tile([P, O], F32)

            kts, qts, vts, msks = {}, {}, {}, {}
            state = {}

            nc.gpsimd.memset(pad_sb[:, 0:32], 0)    # keep the pad slot live

            def load_consts():
                nc.gpsimd.dma_start(wt_sb[:], wtr[:])
                nc.gpsimd.dma_start(bias_sb[:], b_r[:])

            def load_k_chunk(b, c, eng):
                if b not in kts:
                    kts[b] = ktp.tile([P, KT, DT, P], BF, tag="kt", name=f"kt{b}")
                eng.dma_start(
                    kts[b][:, c * 4:(c + 1) * 4, :, :],
                    ktr[b, :, c * 4:(c + 1) * 4, :, :],
                )

            def load_k_fine(b, kt0, nkt, eng):
                if b not in kts:
                    kts[b] = ktp.tile([P, KT, DT, P], BF, tag="kt", name=f"kt{b}")
                eng.dma_start(
                    kts[b][:, kt0:kt0 + nkt, :, :],
                    ktr[b, :, kt0:kt0 + nkt, :, :],
                )

            def load_m_chunk(b, kt0, nkt, eng):
                if b not in msks:
                    msks[b] = mp.tile([P, KT, QC, 512], E5, tag="m", name=f"m{b}")
                eng.dma_start(
                    msks[b][:, kt0:kt0 + nkt, :, :],
                    msk[b, :, kt0:kt0 + nkt, :],
                )

            def load_q(b, qc, eng):
                if b not in qts:
                    qts[b] = qtp.tile([P, QC, DT, 512], BF, tag="qt", name=f"qt{b}")
                eng.dma_start(qts[b][:, qc, :, :], qtr[b, :, qc, :, :])

            def load_v(b, h, eng):
                if b not in vts:
                    vts[b] = vp.tile([P, VT, NK], BF, tag="v", name=f"v{b}")
                eng.dma_start(
                    vts[b][:, :, h * 1024:(h + 1) * 1024],
                    vtr[b, :, :, h * 1024:(h + 1) * 1024],
                )

            def ph1_kt(b, kt, qc):
                """S^T[k, q-chunk] for one (k-tile, q-chunk) -> +maskbias -> exp."""
                if ("et", b) not in state:
                    state[("et", b)] = etp.tile(
                        [P, KT, QC, 512], BF, tag="et", name=f"et{b}"
                    )
                et = state[("et", b)]
                ps = psp1.tile([P, 512], F32, tag="ps1", name=f"ps1_{b}_{kt}_{qc}")
                for dt in range(DT):
                    nc.tensor.matmul(
                        ps[:],
                        kts[b][:, kt, dt, :],
                        qts[b][:, qc, dt, :],
                        start=(dt == 0),
                        stop=(dt == DT - 1),
                    )
                nc.vector.tensor_tensor(
                    ps[:], ps[:], msks[b][:, kt, qc, :], mybir.AluOpType.add,
                )
                nc.scalar.activation(
                    et[:, kt, qc, :], ps[:],
                    mybir.ActivationFunctionType.Exp,
                    scale=float(SCALE),
                )

            def vw_kt(b, kt):
                """VW[k, 0:256] for one k-tile; col 256 is ones."""
                if ("vw", b) not in state:
                    vw = vwp.tile([P, KT, 512], BF, tag="vw", name=f"vw{b}")
                    state[("vw", b)] = vw
                    nc.vector.memset(vw[:, :, O:O + 1], 1.0)
                vw = state[("vw", b)]
                ps = psp2.tile([P, 512], F32, tag="ps2", name=f"psv_{b}_{kt}")
                for vt in range(VT):
                    nc.tensor.matmul(
                        ps[:, :O],
                        vts[b][:, vt, kt * P:(kt + 1) * P],
                        wt_sb[:, vt, :],
                        start=(vt == 0),
                        stop=(vt == VT - 1),
                    )
                nc.scalar.activation(
                    vw[:, kt, 0:O], ps[:, 0:O],
                    mybir.ActivationFunctionType.Copy,
                )

            def ph2_qt(b, qt, eng):
                """ctx[q, 0:256] + den[q] (col 256) for one q-tile; drains and
                stores while the next q-tile's matmuls run (3-deep ps2 ring)."""
                et = state[("et", b)]
                vw = state[("vw", b)]
                if ("osb", b) not in state:
                    state[("osb", b)] = osp.tile([P, QT, O], F32, tag="osb", name=f"o{b}")
                    state[("rc", b)] = dnp.tile([P, 256], F32, tag="rc", name=f"rc{b}")
                out_sb = state[("osb", b)]
                rc = state[("rc", b)]
                ps = psp2.tile([P, 512], F32, tag="ps2", name=f"ps2_{b}_{qt}")
                for kt in range(KT):
                    nc.tensor.matmul(
                        ps[:, 0:O + 1],
                        et[:, kt, qt // 4, (qt % 4) * P:(qt % 4 + 1) * P],
                        vw[:, kt, 0:O + 1],
                        start=(kt == 0),
                        stop=(kt == KT - 1),
                    )
                nc.vector.reciprocal(rc[:, qt:qt + 1], ps[:, O:O + 1])
                nc.vector.scalar_tensor_tensor(
                    out_sb[:, qt, :], ps[:, 0:O],
                    rc[:, qt:qt + 1], bias_sb[:],
                    mybir.AluOpType.mult, mybir.AluOpType.add,
                )
                eng.dma_start(
                    out[b].rearrange("(t p) o -> p t o", p=P)[:, qt:qt + 1, :],
                    out_sb[:, qt:qt + 1, :],
                )

            # ---- schedule
            # The DMA system is a shared ~350-390 GB/s pool split across
            # active queues, so the sweep interleaves qc per 4-kt
            # super-chunk: each K/mask chunk then feeds 6.8us of PE work
            # (both q-halves) instead of 3.4us, halving the early load
            # demand.  Deadlines (PE start ~13.4us): K/M chunk c by
            # 13.4 + 6.8c; Q fully by ~17; V0 loose (VW hoistable by the
            # tile scheduler -> keep V0 arriving by ~28).
            # deadline-ordered round-robin of 0.25MB pieces over the three
            # queues (each ~100GB/s when all active; sync/scalar start
            # ~8.2us, swdge ~11.9us).  PE deadline for kt is 13.4+0.85*kt
            # (K hard, mask +~2us ring slack).
            load_k_fine(0, 0, 4, nc.sync)       # gate: kt0-3      @13.4
            load_q(0, 0, nc.scalar)             # gate: q half0    @13.2
            load_m_chunk(0, 2, 2, nc.gpsimd)    # @14.4
            load_m_chunk(0, 0, 2, nc.scalar)    # @15.7 (need 16.0)
            load_k_fine(0, 4, 2, nc.sync)       # @16.0 (need 16.8)
            load_k_fine(0, 8, 2, nc.gpsimd)     # @16.9 (need 20.2)
            load_k_fine(0, 6, 2, nc.scalar)     # @18.2 (need 18.5)
            load_m_chunk(0, 4, 2, nc.sync)      # @18.6 (need 19.4)
            load_m_chunk(0, 6, 2, nc.gpsimd)    # @19.4 (need 21.1)
            load_m_chunk(0, 8, 2, nc.scalar)    # @20.7 (need 22.8)
            load_k_fine(0, 10, 2, nc.sync)      # @21.2 (need 21.9)
            load_k_fine(0, 12, 2, nc.gpsimd)    # @21.9 (need 23.6)
            load_m_chunk(0, 10, 2, nc.scalar)   # @23.2 (need 24.5)
            load_k_fine(0, 14, 2, nc.sync)      # @23.8 (need 25.3)
            load_m_chunk(0, 12, 2, nc.gpsimd)   # @24.4 (need 26.2)
            load_m_chunk(0, 14, 2, nc.scalar)   # @25.7 (need 27.0)
            load_q(0, 1, nc.gpsimd)             # @26.9 (need 27.1)
            # post-crunch: consts + V0 at the swdge TAIL so the scheduler's
            # simulated completion keeps the VW phase after ph1 (putting V0
            # early on a hwdge queue makes the scheduler hoist VW to PE
            # position ~16, serializing PE on the real v0 arrival)
            load_consts()
            load_v(0, 0, nc.gpsimd)
            load_v(0, 1, nc.gpsimd)

            for kt in range(KT):       # qc0 sweep
                ph1_kt(0, kt, 0)
                if kt == 3:
                    load_k_chunk(1, 0, nc.gpsimd)
                if kt == 7:
                    load_k_chunk(1, 1, nc.gpsimd)
                if kt == 9:
                    load_m_chunk(1, 0, 4, nc.sync)
                if kt == 11:
                    load_m_chunk(1, 4, 4, nc.scalar)
                if kt == 13:
                    load_k_chunk(1, 2, nc.gpsimd)
            for kt in range(8):        # qc1 sweep
                ph1_kt(0, kt, 1)
                if kt == 1:
                    load_k_chunk(1, 3, nc.gpsimd)
                if kt == 3:
                    load_q(1, 0, nc.gpsimd)
                if kt == 5:
                    load_m_chunk(1, 8, 4, nc.sync)
                if kt == 7:
                    load_m_chunk(1, 12, 4, nc.scalar)
            for kt in range(8, KT):
                ph1_kt(0, kt, 1)
                if kt == 10:
                    load_q(1, 1, nc.gpsimd)
            # pin VW after ph1(b0) in the scheduler's simulated timeline —
            # otherwise it hoists VW to PE position ~16-36 and the in-order
            # PE queue serializes on the (late) V0 arrival
            with tc.tile_wait_until(0.030):
                for kt in range(KT):
                    vw_kt(0, kt)

            ph2_qt(0, 0, nc.sync)
            load_v(1, 0, nc.gpsimd)
            ph2_qt(0, 1, nc.sync)
            ph2_qt(0, 2, nc.sync)
            load_v(1, 1, nc.gpsimd)
            for qt in range(3, QT):
                ph2_qt(0, qt, nc.sync)

            for kt in range(KT):
                ph1_kt(1, kt, 0)
            for kt in range(KT):
                ph1_kt(1, kt, 1)
            with tc.tile_wait_until(0.075):
                for kt in range(KT):
                    vw_kt(1, kt)

            for qt in range(6):
                ph2_qt(1, qt, nc.sync)
            ph2_qt(1, 6, nc.sync)
            ph2_qt(1, 7, nc.scalar)

    nc.finalize()
    return nc


def kernel(keys, queries, values, mask, W_resize, b_resize):
    bf = ml_dtypes.bfloat16
    keys = np.asarray(keys, dtype=np.float32)
    queries = np.asarray(queries, dtype=np.float32)
    values = np.asarray(values, dtype=np.float32)
    mask = np.asarray(mask)
    W = np.asarray(W_resize, dtype=np.float32)

    # host-side layout packing (partition-major, contiguous DMA lines)
    # ktr[b, p, kt, dt, kk] = K[b, kt*128+kk, dt*128+p]
    ktr = np.ascontiguousarray(
        keys.reshape(B, KT, P, DT, P).transpose(0, 4, 1, 3, 2)
    ).astype(bf)
    # qtr[b, p, qc, dt, j] = Q[b, qc*512+j, dt*128+p]
    qtr = np.ascontiguousarray(
        queries.reshape(B, QC, 512, DT, P).transpose(0, 4, 1, 3, 2)
    ).astype(bf)
    # vtr[b, p, vt, k] = V[b, k, vt*128+p]
    vtr = np.ascontiguousarray(
        values.reshape(B, NK, VT, P).transpose(0, 3, 2, 1)
    ).astype(bf)
    # mskt[b, p, kt, q] = (mask[b, q, kt*128+p]-1)*28672  (additive bias, fp8e5)
    mb = (mask.astype(np.float32) - 1.0) * 28672.0
    mskt = np.ascontiguousarray(
        mb.reshape(B, NQ, KT, P).transpose(0, 3, 2, 1)
    ).astype(ml_dtypes.float8_e5m2)
    # wtr[p, vt, o] = W^T[vt*128+p, o]
    wtr = np.ascontiguousarray(W.T.reshape(VT, P, O).transpose(1, 0, 2)).astype(bf)
    b_rep = np.ascontiguousarray(
        np.broadcast_to(np.asarray(b_resize, dtype=np.float32).reshape(1, O), (P, O))
    )

    if "nc" not in _NC_CACHE:
        _NC_CACHE["nc"] = _build()
    nc = _NC_CACHE["nc"]

    in_maps = []
    for c in range(N_CORES):
        s = slice(c * B_LOC, (c + 1) * B_LOC)
        in_maps.append(
            {
                "ktr": ktr[s],
                "qtr": qtr[s],
                "vtr": vtr[s],
                "msk": mskt[s],
                "wtr": wtr,
                "b_resize": b_rep,
            }
        )

    global _last_in_maps
    _last_in_maps = in_maps

    r = run_bass_kernel_spmd(nc, in_maps, list(range(N_CORES)))
    return np.concatenate([r.results[c]["out"] for c in range(N_CORES)], axis=0)


_last_in_maps = None
